# revision 6
# baseline (speedup 1.0000x reference)
"""Cross-attention kernel for TRN2, data-parallel over batch (B=8) on 8 cores.

Reference computation per batch element:
    xt  = proj_in(x)              # [L=4096, E=512], 1x1 conv == matmul
    Q   = xt @ W_q.T + b_q
    K   = ctx @ W_k.T + b_k       # ctx: [S=1024, E]
    V   = ctx @ W_v.T + b_v
    att = softmax(Q @ K.T * scale)
    out = proj_out((att @ V).T)   # [C=512, 64, 64]

Host-side algebraic folds (exact up to fp rounding):
  * scale, W_pi, W_q, W_k fold into G = (scale * W_q @ W_pi).T @ W_k, so
    logits = (G @ ctx).T-contract x.  W_v, W_po fold into WV = (W_po @ W_v).T.
  * softmax normalization is applied at the end (multiply by 1/Z).

fp8 attention core with a mean-shift: the logits are tiny (std ~0.09), so
P = exp(s) is ~1.0 everywhere and attention is near-uniform.  Quantizing P
to e4m3 directly would cost ~4% output error; instead the device computes
D = P - 1 (|D| ~ 0.1) and uses the exact decomposition
    U = sum_j VW[j]*P[j] = csum + sum_j VW[j]*D[j],   csum = sum_j VW[j]
    Z = sum_j P[j] = S + sum_j D[j]
csum is a tiny host-side fold of ctx (like the q0 bias path), shipped
replicated; the expensive matmuls (logits ST = GC.T-contract X and
U.T = D.T-contract VW, both 2.1 GMAC/core) run in fp8e4 DoubleRow mode
(2 MACs/cell/cycle).  The one-time GC/VW builder matmuls stay in bf16:
quantizing ctx or G to fp8 leaks a *coherent* error through ctx@ctx.T ~ S*I
(measured 2.7-3.4% output error), while quantizing their outputs GC/VW is
benign (~0.35%).  Measured end-to-end rel err ~1.1e-2 vs the 2e-2 gate.

The U matmul is computed transposed (U.T[i,o], stationary = D-slices) so
the output lands query-major: y DRAM is [L, C] (contiguous 1KB DMA lines)
and the host transposes.  Z rides 4 DR ones-matmuls into [1,512] PSUM;
1/(SW*Z) is broadcast by GpSimd; bf16 output, upcast on host.

Schedule: ~14 bf16 warm-up matmuls on a zero tile keep the PE HAM busy
through the initial DMA (else the first ~21us run at 1.2 GHz), and the
U/Z stage of chunk ic-1 is interleaved into the ST stage of chunk ic so
the PE never waits on the ACT-engine exp chain.
"""

import numpy as np
import ml_dtypes

import concourse.bass as bass
import concourse.mybir as mybir
import concourse.tile as tile
from concourse import bacc
from concourse.bass_utils import run_bass_kernel_spmd

F32 = mybir.dt.float32
BF16 = mybir.dt.bfloat16
F8 = mybir.dt.float8e4
EXP = mybir.ActivationFunctionType.Exp
DR = mybir.MatmulPerfMode.DoubleRow
ADD = mybir.AluOpType.add
MULT = mybir.AluOpType.mult

C = 512       # in channels
E = 512       # emb dim
L = 4096      # query length (64*64)
S = 1024      # key length (32*32)
LI = 512      # query chunk size
NCHUNK = L // LI
NCORES = 8
SG = 2048.0   # host scale on G (keeps e4m3 GC out of the subnormal floor)
SW = 128.0    # host scale on WV
WARM_MMS = 14

TRACE = False           # test harness can flip this before calling kernel()
LAST_RESULTS = None     # stashed BassKernelResults for the test harness

_PROGRAM_CACHE = {}


def _q8(a: np.ndarray) -> np.ndarray:
    return np.clip(np.asarray(a, np.float32), -240.0, 240.0).astype(
        ml_dtypes.float8_e4m3
    )


def _qbf(a: np.ndarray) -> np.ndarray:
    return np.asarray(a, np.float32).astype(ml_dtypes.bfloat16)


def _build_program(has_q0: bool, has_bo: bool):
    nc = bacc.Bacc(
        "TRN2",
        target_bir_lowering=False,
        debug=False,
        enable_asserts=False,
        num_devices=NCORES,
    )
    # host-permuted layouts (see kernel() for the exact index maps)
    x_d = nc.dram_tensor("xq", [128, 4 * L], F8, kind="ExternalInput").ap()
    ctx_d = nc.dram_tensor("ctxb", [128, 4 * S], BF16, kind="ExternalInput").ap()
    gt_d = nc.dram_tensor("gtb", [128, 4 * C], BF16, kind="ExternalInput").ap()
    wv_d = nc.dram_tensor("wvb", [128, 4 * E], BF16, kind="ExternalInput").ap()
    cs_d = nc.dram_tensor("csr", [128, E], F32, kind="ExternalInput").ap()
    q0_d = bo_d = None
    if has_q0:
        q0_d = nc.dram_tensor("q0", [128, 8], F32, kind="ExternalInput").ap()
    if has_bo:
        bo_d = nc.dram_tensor("bo", [128, C], F32, kind="ExternalInput").ap()
    y_d = nc.dram_tensor("y", [L, C], BF16, kind="ExternalOutput").ap()

    with tile.TileContext(nc) as tc:
        from contextlib import ExitStack

        with ExitStack() as ctx:
            cpool = ctx.enter_context(tc.tile_pool(name="consts", bufs=1))
            ps_s = ctx.enter_context(tc.tile_pool(name="ps_s", bufs=3, space="PSUM"))
            ps_u = ctx.enter_context(tc.tile_pool(name="ps_u", bufs=3, space="PSUM"))
            ps_z = ctx.enter_context(tc.tile_pool(name="ps_z", bufs=1, space="PSUM"))
            ppool = ctx.enter_context(tc.tile_pool(name="pp", bufs=6))
            dpool = ctx.enter_context(tc.tile_pool(name="dp", bufs=2))
            opool = ctx.enter_context(tc.tile_pool(name="op", bufs=6))
            zpool = ctx.enter_context(tc.tile_pool(name="zp", bufs=6))

            # ---- warm-up: keep the PE busy through the DMA preamble ------
            wsb = cpool.tile([128, 512], BF16, name="wsb")
            nc.vector.memset(wsb[:], 0.0)
            for k in range(WARM_MMS):
                wps = ps_z.tile([128, 512], F32, name="wps", tag="z")
                nc.tensor.matmul(
                    wps[:], wsb[:, 0:128], wsb[:], start=True, stop=True
                )

            # ---- loads (one big DMA per tensor; queue order = priority) --
            GTS = cpool.tile([128, 4 * C], BF16, name="gts")
            nc.sync.dma_start(GTS[:], gt_d[:, :])
            CTXT = cpool.tile([128, 4 * S], BF16, name="ctxt")
            nc.sync.dma_start(CTXT[:], ctx_d[:, :])
            X8 = cpool.tile([128, 4 * L], F8, name="x8")
            nc.sync.dma_start(X8[:], x_d[:, :])
            WVT = cpool.tile([128, 4 * E], BF16, name="wvt")
            nc.sync.dma_start(WVT[:], wv_d[:, :])
            CSR = cpool.tile([128, E], F32, name="csr")
            nc.sync.dma_start(CSR[:], cs_d[:, :])
            q0_s = bo_s = None
            if has_q0:
                q0_s = cpool.tile([128, 8], F32, name="q0s")
                nc.sync.dma_start(q0_s[:], q0_d[:, :])
            if has_bo:
                bo_s = cpool.tile([128, C], F32, name="bos")
                nc.sync.dma_start(bo_s[:], bo_d[:, :])
            # dual-fp8 LDWEIGHTS requires the pair step to be a multiple of
            # 16 bytes, so the two 1.0 weight columns sit 16 elements apart
            ones2 = cpool.tile([128, 32], F8, name="ones2")
            nc.vector.memset(ones2[:], 1.0)

            GC8 = cpool.tile([128, 4 * S], F8, name="gc8")    # ct-major
            VW8 = cpool.tile([128, 8 * LI], F8, name="vw8")   # jt-major

            # 3D pair views for DoubleRow operands
            gc3 = GC8[:].rearrange("q (c j) -> q c j", c=4)
            x3 = X8[:].rearrange("q (c i) -> q c i", c=4)
            vw3 = VW8[:].rearrange("q (j o) -> q j o", j=8)
            ones3 = ones2[:].rearrange("q (a b) -> q a b", a=2)[:, :, 0:1]

            # ---- GC[c, j] = sum_e G[c, e] ctx[e, j]  (bf16, once) --------
            def gc_group(jh, ct):
                gps = ps_s.tile([128, LI], F32, name="gps", tag="s")
                for et in range(4):
                    nc.tensor.matmul(
                        gps[:],
                        GTS[:, ct * 512 + et * 128: ct * 512 + (et + 1) * 128],
                        CTXT[:, et * S + jh * LI: et * S + (jh + 1) * LI],
                        start=(et == 0),
                        stop=(et == 3),
                    )
                nc.vector.tensor_copy(
                    GC8[:, ct * S + jh * LI: ct * S + (jh + 1) * LI], gps[:]
                )

            for jh in range(2):
                for ct in range(4):
                    gc_group(jh, ct)

            # ---- VW[j, o] = sum_e ctx[e, j] WV[e, o]  (bf16, once) -------
            def vw_group(jt):
                vps = ps_s.tile([128, E], F32, name="vps", tag="s")
                for et in range(4):
                    nc.tensor.matmul(
                        vps[:],
                        CTXT[:, et * S + jt * 128: et * S + (jt + 1) * 128],
                        WVT[:, bass.ts(et, E)],
                        start=(et == 0),
                        stop=(et == 3),
                    )
                nc.vector.tensor_copy(VW8[:, bass.ts(jt, LI)], vps[:])

            # ---- per-chunk stages ---------------------------------------
            def st_group(ic, jt, D8):
                """ST[j,i] = GC.T x X (fp8 DR); P = exp; D = P - 1 (e4m3)."""
                sps = ps_s.tile([128, LI], F32, name="sps", tag="s")
                for p in range(2):
                    nc.tensor.matmul(
                        sps[:],
                        gc3[:, 2 * p: 2 * p + 2, jt * 128:(jt + 1) * 128],
                        x3[:, 2 * p: 2 * p + 2, bass.ts(ic, LI)],
                        start=(p == 0),
                        stop=(p == 1),
                        perf_mode=DR,
                    )
                pt = ppool.tile([128, LI], BF16, name="pt", tag="p")
                if has_q0:
                    nc.scalar.activation(
                        pt[:], sps[:], EXP, bias=q0_s[:, jt:jt + 1], scale=1.0 / SG
                    )
                else:
                    nc.scalar.activation(pt[:], sps[:], EXP, scale=1.0 / SG)
                nc.vector.tensor_scalar_add(D8[:, bass.ts(jt, LI)], pt[:], -1.0)

            def z_stage(d3):
                """Z' = sum_j D[j,i] -> [1, LI]; invz = 1/(SW*(Z'+S)) bf16."""
                zps = ps_z.tile([1, LI], F32, name="zps", tag="z")
                for p in range(4):
                    nc.tensor.matmul(
                        zps[:],
                        ones3[:, :, :],
                        d3[:, 2 * p: 2 * p + 2, :],
                        start=(p == 0),
                        stop=(p == 3),
                        perf_mode=DR,
                    )
                zsb = zpool.tile([1, LI], F32, name="zsb", tag="zsb")
                nc.vector.tensor_scalar(
                    out=zsb[:], in0=zps[:], scalar1=float(S), scalar2=SW,
                    op0=ADD, op1=MULT,
                )
                invz = zpool.tile([1, LI], F32, name="invz", tag="invz")
                nc.vector.reciprocal_approx_fast(out=invz[:], in_=zsb[:])
                invzb = zpool.tile([1, LI], BF16, name="invzb", tag="invzb")
                nc.vector.tensor_copy(invzb[:], invz[:])
                invz_rep = zpool.tile([128, LI], BF16, name="invzr", tag="invzr")
                nc.gpsimd.partition_broadcast(invz_rep[:], invzb[:])
                return invz_rep

            def u_group(ic, s, d3, invz_rep):
                """U.T[i,o] = sum_j D[j,i] VW[j,o] (fp8 DR);
                y = (U.T + csum) * invz -> bf16 -> DRAM [L, C]."""
                ups = ps_u.tile([128, E], F32, name="ups", tag="u")
                for p in range(4):
                    nc.tensor.matmul(
                        ups[:],
                        d3[:, 2 * p: 2 * p + 2, s * 128:(s + 1) * 128],
                        vw3[:, 2 * p: 2 * p + 2, :],
                        start=(p == 0),
                        stop=(p == 3),
                        perf_mode=DR,
                    )
                t = opool.tile([128, E], BF16, name="t", tag=f"t{s}")
                nc.vector.tensor_add(t[:], ups[:], CSR[:])
                o = opool.tile([128, E], BF16, name="o", tag=f"o{s}")
                nc.vector.tensor_mul(o[:], t[:], invz_rep[:])
                if has_bo:
                    nc.vector.tensor_add(o[:], o[:].bitcast(BF16), bo_s[:])
                nc.sync.dma_start(
                    y_d[ic * LI + s * 128: ic * LI + (s + 1) * 128, :], o[:]
                )

            # ---- pipelined emission -------------------------------------
            # chunk 0 ST, with the VW build + casts filling the PE while the
            # ACT exp chain drains; then chunks 1..7 with U/Z of the previous
            # chunk interleaved; then the tail.
            D8_prev = dpool.tile([128, 8 * LI], F8, name="d8", tag="d8")
            for jt in range(8):
                st_group(0, jt, D8_prev)
            for jt in range(8):
                vw_group(jt)

            for ic in range(1, NCHUNK + 1):
                d3_prev = D8_prev[:].rearrange("q (j i) -> q j i", j=8)
                if ic < NCHUNK:
                    D8 = dpool.tile([128, 8 * LI], F8, name="d8", tag="d8")
                    st_group(ic, 0, D8)
                    st_group(ic, 1, D8)
                    invz_rep = z_stage(d3_prev)
                    u_group(ic - 1, 0, d3_prev, invz_rep)
                    st_group(ic, 2, D8)
                    st_group(ic, 3, D8)
                    u_group(ic - 1, 1, d3_prev, invz_rep)
                    st_group(ic, 4, D8)
                    st_group(ic, 5, D8)
                    u_group(ic - 1, 2, d3_prev, invz_rep)
                    st_group(ic, 6, D8)
                    st_group(ic, 7, D8)
                    u_group(ic - 1, 3, d3_prev, invz_rep)
                    D8_prev = D8
                else:
                    invz_rep = z_stage(d3_prev)
                    for s in range(4):
                        u_group(ic - 1, s, d3_prev, invz_rep)

    nc.compile()
    return nc


def kernel(**inputs) -> np.ndarray:
    global LAST_RESULTS
    x = np.asarray(inputs["x"], dtype=np.float32)
    context = np.asarray(inputs["context"], dtype=np.float32)
    W_pi = np.asarray(inputs["W_pi"], dtype=np.float64)
    b_pi = np.asarray(inputs["b_pi"], dtype=np.float64)
    W_q = np.asarray(inputs["W_q"], dtype=np.float64)
    b_q = np.asarray(inputs["b_q"], dtype=np.float64)
    W_k = np.asarray(inputs["W_k"], dtype=np.float64)
    W_v = np.asarray(inputs["W_v"], dtype=np.float64)
    b_v = np.asarray(inputs["b_v"], dtype=np.float64)
    W_po = np.asarray(inputs["W_po"], dtype=np.float64)
    b_po = np.asarray(inputs["b_po"], dtype=np.float64)

    scale = float(E) ** -0.5
    G = (scale * (W_q @ W_pi)).T @ W_k                     # [c, e]
    WV = (W_po @ W_v).T                                    # [e, o]
    # gt layout: gtb[p, ct*512+et*128+c'] = (SG*G).T[et*128+p, ct*128+c']
    GT = np.ascontiguousarray((SG * G).T.astype(np.float32))
    gtb = _qbf(
        GT.reshape(4, 128, 4, 128).transpose(1, 2, 0, 3).reshape(128, 4 * C)
    )
    # wv layout: wvb[p, et*512+o] = (SW*WV)[et*128+p, o]
    wvb = _qbf((SW * WV).astype(np.float32).reshape(4, 128, E)
               .transpose(1, 0, 2).reshape(128, 4 * E))
    b_row = scale * (W_q @ b_pi + b_q)
    q0_e = (W_k.T @ b_row).astype(np.float64)              # [e]
    b_o = (b_po + W_po @ b_v).astype(np.float32)           # [o]

    has_q0 = bool(np.any(q0_e))
    has_bo = bool(np.any(b_o))
    key = (has_q0, has_bo)
    if key not in _PROGRAM_CACHE:
        _PROGRAM_CACHE[key] = _build_program(has_q0, has_bo)
    nc = _PROGRAM_CACHE[key]

    in_maps = []
    for c in range(NCORES):
        ctx_mat = context[c].reshape(E, S)
        xm = x[c].reshape(C, L)
        # xq[p, ct*4096+i] = x[ct*128+p, i]
        xq = _q8(xm.reshape(4, 128, L).transpose(1, 0, 2).reshape(128, 4 * L))
        # ctxb[p, et*1024+j] = ctx[et*128+p, j]
        ctxb = _qbf(ctx_mat.reshape(4, 128, S).transpose(1, 0, 2)
                    .reshape(128, 4 * S))
        # csum (SW-scaled), replicated across partitions
        csum = (ctx_mat.astype(np.float64).sum(axis=1) @ (SW * WV)).astype(
            np.float32
        )
        csr = np.ascontiguousarray(
            np.broadcast_to(csum[None, :], (128, E)), dtype=np.float32
        )
        m = {"xq": xq, "ctxb": ctxb, "gtb": gtb, "wvb": wvb, "csr": csr}
        if has_q0:
            q0j = (q0_e @ ctx_mat.astype(np.float64)).astype(np.float32)
            m["q0"] = np.ascontiguousarray(q0j.reshape(8, 128).T)
        if has_bo:
            m["bo"] = np.ascontiguousarray(
                np.broadcast_to(b_o[None, :], (128, C)), dtype=np.float32
            )
        in_maps.append(m)

    res = run_bass_kernel_spmd(nc, in_maps, core_ids=list(range(NCORES)), trace=TRACE)
    LAST_RESULTS = res
    y = np.stack(
        [
            np.asarray(res.results[c]["y"]).astype(np.float32).T.reshape(C, 64, 64)
            for c in range(NCORES)
        ],
        axis=0,
    )
    return np.ascontiguousarray(y)


# revision 8
# speedup vs baseline: 1.1793x; 1.1793x over previous
"""Cross-attention kernel for TRN2, data-parallel over batch (B=8) on 8 cores.

Reference computation per batch element:
    xt  = proj_in(x)              # [L=4096, E=512], 1x1 conv == matmul
    Q   = xt @ W_q.T + b_q
    K   = ctx @ W_k.T + b_k       # ctx: [S=1024, E]
    V   = ctx @ W_v.T + b_v
    att = softmax(Q @ K.T * scale)
    out = proj_out((att @ V).T)   # [C=512, 64, 64]

Host-side algebraic folds (exact up to fp rounding):
  * scale, W_pi, W_q, W_k fold into G = (scale * W_q @ W_pi).T @ W_k, so
    logits = (G @ ctx).T-contract x.  W_v, W_po fold into WV = (W_po @ W_v).T.
  * softmax normalization is applied at the end (multiply by 1/Z).

fp8 attention core with a mean-shift: the logits are tiny (std ~0.09), so
P = exp(s) is ~1.0 everywhere and attention is near-uniform.  Quantizing P
to e4m3 directly would cost ~4% output error; instead the device computes
D = P - 1 (|D| ~ 0.1) and uses the exact decomposition
    U = sum_j VW[j]*P[j] = csum + sum_j VW[j]*D[j],   csum = sum_j VW[j]
    Z = sum_j P[j] = S + sum_j D[j]
csum is a tiny host-side fold of ctx (like the q0 bias path), shipped
replicated; the expensive matmuls (logits ST = GC.T-contract X and
U.T = D.T-contract VW, both 2.1 GMAC/core) run in fp8e4 DoubleRow mode
(2 MACs/cell/cycle).  The one-time GC/VW builder matmuls stay in bf16:
quantizing ctx or G to fp8 leaks a *coherent* error through ctx@ctx.T ~ S*I
(measured 2.7-3.4% output error), while quantizing their outputs GC/VW is
benign (~0.35%).  Measured end-to-end rel err ~1.1e-2 vs the 2e-2 gate.

The U matmul is computed transposed (U.T[i,o], stationary = D-slices) so
the output lands query-major: y DRAM is [L, C] (contiguous 1KB DMA lines)
and the host transposes.  Z rides 4 DR ones-matmuls into [1,512] PSUM;
1/(SW*Z) is broadcast by GpSimd; bf16 output, upcast on host.

Schedule: ~14 bf16 warm-up matmuls on a zero tile keep the PE HAM busy
through the initial DMA (else the first ~21us run at 1.2 GHz), and the
U/Z stage of chunk ic-1 is interleaved into the ST stage of chunk ic so
the PE never waits on the ACT-engine exp chain.
"""

import numpy as np
import ml_dtypes

import concourse.bass as bass
import concourse.mybir as mybir
import concourse.tile as tile
from concourse import bacc
from concourse.bass_utils import run_bass_kernel_spmd

F32 = mybir.dt.float32
BF16 = mybir.dt.bfloat16
F16 = mybir.dt.float16
F8 = mybir.dt.float8e4
EXP = mybir.ActivationFunctionType.Exp
DR = mybir.MatmulPerfMode.DoubleRow
ADD = mybir.AluOpType.add
MULT = mybir.AluOpType.mult

C = 512       # in channels
E = 512       # emb dim
L = 4096      # query length (64*64)
S = 1024      # key length (32*32)
LI = 512      # query chunk size
NCHUNK = L // LI
NCORES = 8
SG = 2048.0   # host scale on G (keeps e4m3 GC out of the subnormal floor)
SW = 128.0    # host scale on WV
WARM_MMS = 14

TRACE = False           # test harness can flip this before calling kernel()
LAST_RESULTS = None     # stashed BassKernelResults for the test harness

_PROGRAM_CACHE = {}


def _q8(a: np.ndarray) -> np.ndarray:
    return np.clip(np.asarray(a, np.float32), -240.0, 240.0).astype(
        ml_dtypes.float8_e4m3
    )


def _qbf(a: np.ndarray) -> np.ndarray:
    return np.asarray(a, np.float32).astype(np.float16)


def _build_program(has_q0: bool, has_bo: bool):
    nc = bacc.Bacc(
        "TRN2",
        target_bir_lowering=False,
        debug=False,
        enable_asserts=False,
        num_devices=NCORES,
    )
    # host-permuted layouts (see kernel() for the exact index maps)
    x_d = nc.dram_tensor("xq", [128, 4 * L], F8, kind="ExternalInput").ap()
    ctx_d = nc.dram_tensor("ctxb", [128, 4 * S], F16, kind="ExternalInput").ap()
    gt_d = nc.dram_tensor("gtb", [128, 4 * C], F16, kind="ExternalInput").ap()
    wv_d = nc.dram_tensor("wvb", [128, 4 * E], F16, kind="ExternalInput").ap()
    cs_d = nc.dram_tensor("csr", [128, 4], F32, kind="ExternalInput").ap()
    q0_d = bo_d = None
    if has_q0:
        q0_d = nc.dram_tensor("q0", [128, 8], F32, kind="ExternalInput").ap()
    if has_bo:
        bo_d = nc.dram_tensor("bo", [128, C], F32, kind="ExternalInput").ap()
    y_d = nc.dram_tensor("y", [C, L], F16, kind="ExternalOutput").ap()

    with tile.TileContext(nc) as tc:
        from contextlib import ExitStack

        with ExitStack() as ctx:
            cpool = ctx.enter_context(tc.tile_pool(name="consts", bufs=1))
            ps_s = ctx.enter_context(tc.tile_pool(name="ps_s", bufs=3, space="PSUM"))
            ps_u = ctx.enter_context(tc.tile_pool(name="ps_u", bufs=3, space="PSUM"))
            ps_z = ctx.enter_context(tc.tile_pool(name="ps_z", bufs=2, space="PSUM"))
            ppool = ctx.enter_context(tc.tile_pool(name="pp", bufs=6))
            dpool = ctx.enter_context(tc.tile_pool(name="dp", bufs=2))
            opool = ctx.enter_context(tc.tile_pool(name="op", bufs=6))
            zpool = ctx.enter_context(tc.tile_pool(name="zp", bufs=6))

            # ---- loads (one big DMA per tensor; queue order = priority) --
            GTS = cpool.tile([128, 4 * C], F16, name="gts")
            nc.sync.dma_start(GTS[:], gt_d[:, :])
            CTXT = cpool.tile([128, 4 * S], F16, name="ctxt")
            nc.sync.dma_start(CTXT[:], ctx_d[:, :])
            X8 = cpool.tile([128, 4 * L], F8, name="x8")
            nc.sync.dma_start(X8[:], x_d[:, :])
            WVT = cpool.tile([128, 4 * E], F16, name="wvt")
            nc.sync.dma_start(WVT[:], wv_d[:, :])
            CSR = cpool.tile([128, 4], F32, name="csr")
            nc.sync.dma_start(CSR[:], cs_d[:, :])
            q0_s = bo_s = None
            if has_q0:
                q0_s = cpool.tile([128, 8], F32, name="q0s")
                nc.sync.dma_start(q0_s[:], q0_d[:, :])
            if has_bo:
                bo_s = cpool.tile([128, C], F32, name="bos")
                nc.sync.dma_start(bo_s[:], bo_d[:, :])
            # dual-fp8 LDWEIGHTS requires the pair step to be a multiple of
            # 16 bytes, so the two 1.0 weight columns sit 16 elements apart
            ones2 = cpool.tile([128, 32], F8, name="ones2")
            nc.vector.memset(ones2[:], 1.0)

            GC8 = cpool.tile([128, 4 * S], F8, name="gc8")    # ct-major
            VW8 = cpool.tile([128, 8 * LI], F8, name="vw8")   # jt-major

            # 3D pair views for DoubleRow operands
            gc3 = GC8[:].rearrange("q (c j) -> q c j", c=4)
            x3 = X8[:].rearrange("q (c i) -> q c i", c=4)
            vw3 = VW8[:].rearrange("q (j o) -> q j o", j=8)
            ones3 = ones2[:].rearrange("q (a b) -> q a b", a=2)[:, :, 0:1]

            # ---- GC[c, j] = sum_e G[c, e] ctx[e, j]  (bf16, once) --------
            def gc_group(jh, ct):
                gps = ps_s.tile([128, LI], F32, name="gps", tag="s")
                for et in range(4):
                    nc.tensor.matmul(
                        gps[:],
                        GTS[:, ct * 512 + et * 128: ct * 512 + (et + 1) * 128],
                        CTXT[:, et * S + jh * LI: et * S + (jh + 1) * LI],
                        start=(et == 0),
                        stop=(et == 3),
                    )
                nc.vector.tensor_copy(
                    GC8[:, ct * S + jh * LI: ct * S + (jh + 1) * LI], gps[:]
                )

            for jh in range(2):
                for ct in range(4):
                    gc_group(jh, ct)

            # ---- VW[j, o] = sum_e ctx[e, j] WV[e, o]  (bf16, once) -------
            def vw_group(jt):
                vps = ps_s.tile([128, E], F32, name="vps", tag="s")
                for et in range(4):
                    nc.tensor.matmul(
                        vps[:],
                        CTXT[:, et * S + jt * 128: et * S + (jt + 1) * 128],
                        WVT[:, bass.ts(et, E)],
                        start=(et == 0),
                        stop=(et == 3),
                    )
                nc.vector.tensor_copy(VW8[:, bass.ts(jt, LI)], vps[:])

            # ---- per-chunk stages ---------------------------------------
            def st_group(ic, jt, D8):
                """ST[j,i] = GC.T x X (fp8 DR); P = exp; D = P - 1 (e4m3)."""
                sps = ps_s.tile([128, LI], F32, name="sps", tag="s")
                for p in range(2):
                    nc.tensor.matmul(
                        sps[:],
                        gc3[:, 2 * p: 2 * p + 2, jt * 128:(jt + 1) * 128],
                        x3[:, 2 * p: 2 * p + 2, bass.ts(ic, LI)],
                        start=(p == 0),
                        stop=(p == 1),
                        perf_mode=DR,
                    )
                pt = ppool.tile([128, LI], F16, name="pt", tag="p")
                if has_q0:
                    nc.scalar.activation(
                        pt[:], sps[:], EXP, bias=q0_s[:, jt:jt + 1], scale=1.0 / SG
                    )
                else:
                    nc.scalar.activation(pt[:], sps[:], EXP, scale=1.0 / SG)
                nc.vector.tensor_scalar_add(D8[:, bass.ts(jt, LI)], pt[:], -1.0)

            def z_stage(d3):
                """Z' = sum_j D[j,i] -> [1, LI]; invz = 1/(SW*(Z'+S)) bf16."""
                zps = ps_z.tile([1, LI], F32, name="zps", tag="z")
                for p in range(4):
                    nc.tensor.matmul(
                        zps[:],
                        ones3[:, :, :],
                        d3[:, 2 * p: 2 * p + 2, :],
                        start=(p == 0),
                        stop=(p == 3),
                        perf_mode=DR,
                    )
                zsb = zpool.tile([1, LI], F32, name="zsb", tag="zsb")
                nc.vector.tensor_scalar(
                    out=zsb[:], in0=zps[:], scalar1=float(S), scalar2=SW / 4096.0,
                    op0=ADD, op1=MULT,
                )
                invz = zpool.tile([1, LI], F32, name="invz", tag="invz")
                nc.vector.reciprocal_approx_fast(out=invz[:], in_=zsb[:])
                invzb = zpool.tile([1, LI], F16, name="invzb", tag="invzb")
                nc.vector.tensor_copy(invzb[:], invz[:])
                invz_rep = zpool.tile([128, LI], F16, name="invzr", tag="invzr")
                nc.gpsimd.partition_broadcast(invz_rep[:], invzb[:])
                return invz_rep

            def u_group(ic, s, d3, invz_rep):
                """U[o,i] = sum_j VW[j,o] D[j,i] (fp8 DR);
                y = (U + csum[o])/4096 * invz -> fp16 -> DRAM [C, L]."""
                ups = ps_u.tile([128, LI], F32, name="ups", tag="u")
                for p in range(4):
                    nc.tensor.matmul(
                        ups[:],
                        vw3[:, 2 * p: 2 * p + 2, s * 128:(s + 1) * 128],
                        d3[:, 2 * p: 2 * p + 2, :],
                        start=(p == 0),
                        stop=(p == 3),
                        perf_mode=DR,
                    )
                t = opool.tile([128, LI], F16, name="t", tag=f"t{s}")
                nc.vector.tensor_scalar(
                    out=t[:], in0=ups[:], scalar1=CSR[:, s:s + 1],
                    scalar2=1.0 / 4096.0, op0=ADD, op1=MULT,
                )
                o = opool.tile([128, LI], F16, name="o", tag=f"o{s}")
                nc.vector.tensor_mul(o[:], t[:], invz_rep[:])
                if has_bo:
                    nc.vector.tensor_add(o[:], o[:].bitcast(F16), bo_s[:, s * 128:(s + 1) * 128])
                nc.sync.dma_start(
                    y_d[s * 128:(s + 1) * 128, bass.ts(ic, LI)], o[:]
                )

            # ---- pipelined emission -------------------------------------
            # chunk 0 ST, with the VW build + casts filling the PE while the
            # ACT exp chain drains; then chunks 1..7 with U/Z of the previous
            # chunk interleaved; then the tail.
            D8_prev = dpool.tile([128, 8 * LI], F8, name="d8", tag="d8")
            for jt in range(8):
                st_group(0, jt, D8_prev)
            for jt in range(8):
                vw_group(jt)

            for ic in range(1, NCHUNK + 1):
                d3_prev = D8_prev[:].rearrange("q (j i) -> q j i", j=8)
                if ic < NCHUNK:
                    D8 = dpool.tile([128, 8 * LI], F8, name="d8", tag="d8")
                    st_group(ic, 0, D8)
                    st_group(ic, 1, D8)
                    invz_rep = z_stage(d3_prev)
                    u_group(ic - 1, 0, d3_prev, invz_rep)
                    st_group(ic, 2, D8)
                    st_group(ic, 3, D8)
                    u_group(ic - 1, 1, d3_prev, invz_rep)
                    st_group(ic, 4, D8)
                    st_group(ic, 5, D8)
                    u_group(ic - 1, 2, d3_prev, invz_rep)
                    st_group(ic, 6, D8)
                    st_group(ic, 7, D8)
                    u_group(ic - 1, 3, d3_prev, invz_rep)
                    D8_prev = D8
                else:
                    invz_rep = z_stage(d3_prev)
                    for s in range(4):
                        u_group(ic - 1, s, d3_prev, invz_rep)

    nc.compile()
    return nc


def kernel(**inputs) -> np.ndarray:
    global LAST_RESULTS
    x = np.asarray(inputs["x"], dtype=np.float32)
    context = np.asarray(inputs["context"], dtype=np.float32)
    W_pi = np.asarray(inputs["W_pi"], dtype=np.float64)
    b_pi = np.asarray(inputs["b_pi"], dtype=np.float64)
    W_q = np.asarray(inputs["W_q"], dtype=np.float64)
    b_q = np.asarray(inputs["b_q"], dtype=np.float64)
    W_k = np.asarray(inputs["W_k"], dtype=np.float64)
    W_v = np.asarray(inputs["W_v"], dtype=np.float64)
    b_v = np.asarray(inputs["b_v"], dtype=np.float64)
    W_po = np.asarray(inputs["W_po"], dtype=np.float64)
    b_po = np.asarray(inputs["b_po"], dtype=np.float64)

    scale = float(E) ** -0.5
    G = (scale * (W_q @ W_pi)).T @ W_k                     # [c, e]
    WV = (W_po @ W_v).T                                    # [e, o]
    # gt layout: gtb[p, ct*512+et*128+c'] = (SG*G).T[et*128+p, ct*128+c']
    GT = np.ascontiguousarray((SG * G).T.astype(np.float32))
    gtb = _qbf(
        GT.reshape(4, 128, 4, 128).transpose(1, 2, 0, 3).reshape(128, 4 * C)
    )
    # wv layout: wvb[p, et*512+o] = (SW*WV)[et*128+p, o]
    wvb = _qbf((SW * WV).astype(np.float32).reshape(4, 128, E)
               .transpose(1, 0, 2).reshape(128, 4 * E))
    b_row = scale * (W_q @ b_pi + b_q)
    q0_e = (W_k.T @ b_row).astype(np.float64)              # [e]
    b_o = (b_po + W_po @ b_v).astype(np.float32)           # [o]

    has_q0 = bool(np.any(q0_e))
    has_bo = bool(np.any(b_o))
    key = (has_q0, has_bo)
    if key not in _PROGRAM_CACHE:
        _PROGRAM_CACHE[key] = _build_program(has_q0, has_bo)
    nc = _PROGRAM_CACHE[key]

    in_maps = []
    for c in range(NCORES):
        ctx_mat = context[c].reshape(E, S)
        xm = x[c].reshape(C, L)
        # xq[p, ct*4096+i] = x[ct*128+p, i]
        xq = _q8(xm.reshape(4, 128, L).transpose(1, 0, 2).reshape(128, 4 * L))
        # ctxb[p, et*1024+j] = ctx[et*128+p, j]
        ctxb = _qbf(ctx_mat.reshape(4, 128, S).transpose(1, 0, 2)
                    .reshape(128, 4 * S))
        # csum (SW-scaled), replicated across partitions
        csum = (ctx_mat.astype(np.float64).sum(axis=1) @ (SW * WV)).astype(
            np.float32
        )
        csr = np.ascontiguousarray(csum.reshape(4, 128).T, dtype=np.float32)
        m = {"xq": xq, "ctxb": ctxb, "gtb": gtb, "wvb": wvb, "csr": csr}
        if has_q0:
            q0j = (q0_e @ ctx_mat.astype(np.float64)).astype(np.float32)
            m["q0"] = np.ascontiguousarray(q0j.reshape(8, 128).T)
        if has_bo:
            m["bo"] = np.ascontiguousarray(
                np.broadcast_to(b_o[None, :], (128, C)), dtype=np.float32
            )
        in_maps.append(m)

    res = run_bass_kernel_spmd(nc, in_maps, core_ids=list(range(NCORES)), trace=TRACE)
    LAST_RESULTS = res
    y = np.stack(
        [
            np.asarray(res.results[c]["y"]).astype(np.float32).reshape(C, 64, 64)
            for c in range(NCORES)
        ],
        axis=0,
    )
    return np.ascontiguousarray(y)
